# revision 1
# baseline (speedup 1.0000x reference)
"""Trainium2 Bass kernel for a 4-layer dense transformer (CompressiveTransformer).

Model: V=32000, D=1024, L=4, H=8, B=2, T=2048.

Sharding: the reference attention reshapes (B, T//H, H, D) -> heads are the
strided token residue classes t mod 8.  With 8 cores, core c owns tokens
{t : t % 8 == c} of both batch rows (512 tokens).  Attention, layernorm, FFN
and logits are then all fully core-local: no collectives.

On-chip layout: the residual stream h is kept feature-major
[128 part, 8 dchunk, 512 tok] fp32 in SBUF for the whole kernel.  Matmuls run
in bf16 (fp32 PSUM accumulation); weights are cast to bf16 on the host and
uploaded in column-blocked layout so weight-tile DMAs are contiguous.
"""

import numpy as np
import ml_dtypes

import concourse.bass as bass
import concourse.mybir as mybir
import concourse.tile as tile
from concourse import bacc
from concourse.bass_utils import run_bass_kernel_spmd
from concourse.masks import make_identity

# ---------------------------------------------------------------- constants
V = 32000
D = 1024
L = 4
H = 8
B = 2
T = 2048
NC = 8                     # cores
TLOC = B * T // NC         # 512 tokens per core
P = 128
DC = D // P                # 8 feature chunks
FC = 4 * D // P            # 32 ff chunks
TT = TLOC // P             # 4 token tiles per core
GRP = B                    # attention groups per core (one per batch row)
GT = T // H                # 256 tokens per attention group
VS = 2000                  # vocab super-chunk
NVS = V // VS              # 16
SCALE = float((D // H) ** -0.5)
EPS = 1e-5
NEG_SLOPE = 0.01

FP32 = mybir.dt.float32
BF16 = mybir.dt.bfloat16
I16 = mybir.dt.int16
AF = mybir.ActivationFunctionType
ALU = mybir.AluOpType
AX = mybir.AxisListType

_CACHE = {}


# ---------------------------------------------------------------- emission
def _emit(nc, tc):
    singles = tc.alloc_tile_pool(name="singles", bufs=1)
    ident_f32 = singles.tile([P, P], FP32, name="ident_f32")
    make_identity(nc, ident_f32)
    ident_bf = singles.tile([P, P], BF16, name="ident_bf")
    make_identity(nc, ident_bf)
    ones_col = singles.tile([P, 1], BF16, name="ones_col")  # lhsT for colsums
    nc.vector.memset(ones_col, 1.0)
    eps_sb = singles.tile([1, 1], FP32, name="eps_sb")
    nc.vector.memset(eps_sb, EPS)

    # per-feature params, fm layout [128, L, DC]
    def param_sb(name, dram, nchunk):
        t = singles.tile([P, L, nchunk], FP32, name=name)
        nc.sync.dma_start(t[:], dram.ap().rearrange("l (c p) -> p l c", p=P))
        return t

    ln1g = param_sb("ln1g", nc._dram["ln1_g"], DC)
    ln1b = param_sb("ln1b", nc._dram["ln1_b"], DC)
    ln2g = param_sb("ln2g", nc._dram["ln2_g"], DC)
    ln2b = param_sb("ln2b", nc._dram["ln2_b"], DC)
    woutb = param_sb("woutb", nc._dram["wout_b"], DC)
    ff2b = param_sb("ff2b", nc._dram["ff2_b"], DC)
    ff1b = param_sb("ff1b", nc._dram["ff1_b"], FC)

    idx_sb = singles.tile([P, TLOC // 16], I16, name="idx_sb")
    nc.sync.dma_start(idx_sb[:], nc._dram["idx"].ap())

    resid = tc.alloc_tile_pool(name="resid", bufs=1)
    h = resid.tile([P, DC, TLOC], FP32, name="h")

    # ---------------- phase 0: embedding gather + transpose to fm ----------
    with tc.tile_pool(name="embp", bufs=1) as embp, \
         tc.tile_pool(name="embps", bufs=4, space="PSUM") as embps:
        emb_tm = embp.tile([P, TT, D], FP32, name="emb_tm")
        nc.gpsimd.dma_gather(
            out_ap=emb_tm[:],
            in_ap=nc._dram["emb"].ap(),
            idxs_ap=idx_sb[:],
            num_idxs=TLOC,
            num_idxs_reg=TLOC,
            elem_size=D,
        )
        for c in range(TT):
            for dc in range(DC):
                ps = embps.tile([P, P], FP32, tag="tp")
                nc.tensor.transpose(ps[:], emb_tm[:, c, dc * P:(dc + 1) * P],
                                    ident_f32[:])
                nc.vector.tensor_copy(out=h[:, dc, c * P:(c + 1) * P], in_=ps[:])

    # ---------------- transformer layers ----------------------------------
    acts = tc.alloc_tile_pool(name="acts", bufs=1)
    wst = tc.alloc_tile_pool(name="wst", bufs=1)
    ps_pool = tc.alloc_tile_pool(name="ps", bufs=1, space="PSUM")
    lnp = tc.alloc_tile_pool(name="lnp", bufs=1)

    def layernorm(li, gain, bias_, y):
        """y = LN(h) * gain + bias  (bf16, fm layout), stats over features."""
        hb = lnp.tile([P, DC, TLOC], BF16, tag="hb")
        h2 = lnp.tile([P, DC, TLOC], BF16, tag="h2")
        # per-chunk casts so colsums can start while the previous matmul
        # phase is still finalizing other h chunks
        for dc in range(DC):
            nc.scalar.copy(out=hb[:, dc, :], in_=h[:, dc, :])
            nc.vector.tensor_mul(out=h2[:, dc, :], in0=h[:, dc, :],
                                 in1=h[:, dc, :])
        ps_s = ps_pool.tile([1, TLOC], FP32, tag="stat", bufs=2)
        ps_s2 = ps_pool.tile([1, TLOC], FP32, tag="stat", bufs=2)
        for dc in range(DC):
            nc.tensor.matmul(ps_s[:], ones_col[:], hb[:, dc, :],
                             start=(dc == 0), stop=(dc == DC - 1))
        for dc in range(DC):
            nc.tensor.matmul(ps_s2[:], ones_col[:], h2[:, dc, :],
                             start=(dc == 0), stop=(dc == DC - 1))
        mr = lnp.tile([1, 2, TLOC], FP32, tag="mr", bufs=1)  # mu | rs packed
        nc.vector.tensor_scalar_mul(mr[:, 0, :], ps_s[:], 1.0 / D)
        mumu = lnp.tile([1, TLOC], FP32, tag="mumu", bufs=1)
        nc.vector.tensor_mul(out=mumu[:], in0=mr[:, 0, :], in1=mr[:, 0, :])
        var = lnp.tile([1, TLOC], FP32, tag="var", bufs=1)
        nc.vector.scalar_tensor_tensor(
            out=var[:], in0=ps_s2[:], scalar=1.0 / D, in1=mumu[:],
            op0=ALU.mult, op1=ALU.subtract)
        lnv = lnp.tile([1, TLOC], FP32, tag="lnv", bufs=1)
        nc.scalar.activation(out=lnv[:], in_=var[:], func=AF.Ln, bias=eps_sb[:])
        nc.scalar.activation(out=mr[:, 1, :], in_=lnv[:], func=AF.Exp,
                             scale=-0.5)
        mr_b = lnp.tile([P, 2, TLOC], FP32, tag="mr_b", bufs=1)
        nc.gpsimd.partition_broadcast(mr_b[:], mr[:])
        for dc in range(DC):
            t1 = lnp.tile([P, TLOC], FP32, tag="t1", bufs=2)
            nc.vector.tensor_sub(out=t1[:], in0=h[:, dc, :], in1=mr_b[:, 0, :])
            t2 = lnp.tile([P, TLOC], FP32, tag="t2", bufs=2)
            nc.vector.tensor_mul(out=t2[:], in0=t1[:], in1=mr_b[:, 1, :])
            nc.scalar.activation(
                out=y[:, dc, :], in_=t2[:], func=AF.Identity,
                scale=gain[:, li, dc:dc + 1], bias=bias_[:, li, dc:dc + 1])

    wqkv_d = nc._dram["wqkv"]   # [L, 24, D, 128] blocked bf16
    wout_d = nc._dram["wout"]   # [L, 8, D, 128]
    ff1_d = nc._dram["ff1w"]    # [L, 32, D, 128]
    ff2_d = nc._dram["ff2w"]    # [L, 8, 4D, 128]

    def load_wcol(dram, li, col, kc):
        """Host-swizzled weight column [P, kc, 128] -> SBUF, contiguous."""
        t = wst.tile([P, kc, P], BF16, tag=f"wcol{kc}", bufs=4 if kc == DC else 3)
        nc.sync.dma_start(t[:], dram.ap()[li, col])
        return t

    for li in range(L):
        # ---- LN1 -> y
        y = acts.tile([P, DC, TLOC], BF16, tag="y")
        layernorm(li, ln1g, ln1b, y)

        # ---- q, k fm via weight-stationary matmuls
        q = acts.tile([P, DC, TLOC], BF16, tag="q")
        k = acts.tile([P, DC, TLOC], BF16, tag="k")
        for dst, base in ((q, 0), (k, DC)):
            for m in range(DC):
                w = load_wcol(wqkv_d, li, base + m, DC)
                ps = ps_pool.tile([P, TLOC], FP32, tag="mm", bufs=4)
                for kc in range(DC):
                    nc.tensor.matmul(ps[:], w[:, kc, :], y[:, kc, :],
                                     start=(kc == 0), stop=(kc == DC - 1))
                nc.vector.tensor_copy(out=dst[:, m, :], in_=ps[:])

        # ---- v token-major via activation-stationary matmuls
        v = acts.tile([P, TT, D], BF16, tag="v")
        for half in range(2):
            wv = wst.tile([P, DC, 512], BF16, tag="wv", bufs=2)
            for cb in range(4):
                nc.sync.dma_start(
                    wv[:, :, cb * P:(cb + 1) * P],
                    wqkv_d.ap()[li, 2 * DC + 4 * half + cb])
            for tt in range(TT):
                ps = ps_pool.tile([P, 512], FP32, tag="mm", bufs=4)
                for kc in range(DC):
                    nc.tensor.matmul(ps[:], y[:, kc, tt * P:(tt + 1) * P],
                                     wv[:, kc, :],
                                     start=(kc == 0), stop=(kc == DC - 1))
                nc.vector.tensor_copy(
                    out=v[:, tt, half * 512:(half + 1) * 512], in_=ps[:])

        # ---- attention per group (fully local, 256 tokens, head dim D)
        o = acts.tile([P, DC, TLOC], BF16, tag="o")
        for g in range(GRP):
            attnT = acts.tile([P, 2, GT], BF16, tag="attnT")
            for it in range(2):
                ps_d = ps_pool.tile([P, GT], FP32, tag="mm", bufs=4)
                for kc in range(DC):
                    nc.tensor.matmul(
                        ps_d[:], q[:, kc, (2 * g + it) * P:(2 * g + it + 1) * P],
                        k[:, kc, g * GT:(g + 1) * GT],
                        start=(kc == 0), stop=(kc == DC - 1))
                mx = lnp.tile([P, 1], FP32, tag="mx", bufs=2)
                nc.vector.reduce_max(mx[:], ps_d[:], axis=AX.X)
                nmx = lnp.tile([P, 1], FP32, tag="nmx", bufs=2)
                nc.vector.tensor_scalar_mul(nmx[:], mx[:], -SCALE)
                ae = lnp.tile([P, GT], BF16, tag="ae", bufs=2)
                se = lnp.tile([P, 1], FP32, tag="se", bufs=2)
                nc.scalar.activation(out=ae[:], in_=ps_d[:], func=AF.Exp,
                                     bias=nmx[:], scale=SCALE, accum_out=se[:])
                rse = lnp.tile([P, 1], FP32, tag="rse", bufs=2)
                nc.vector.reciprocal(out=rse[:], in_=se[:])
                an = lnp.tile([P, GT], BF16, tag="an", bufs=2)
                nc.vector.tensor_scalar_mul(an[:], ae[:], rse[:])
                for jc in range(2):
                    ps_t = ps_pool.tile([P, P], BF16, tag="tp", bufs=2)
                    nc.tensor.transpose(ps_t[:], an[:, jc * P:(jc + 1) * P],
                                        ident_bf[:])
                    nc.vector.tensor_copy(out=attnT[:, jc, it * P:(it + 1) * P],
                                          in_=ps_t[:])
            for m in range(DC):
                ps_o = ps_pool.tile([P, GT], FP32, tag="mm", bufs=4)
                for jc in range(2):
                    nc.tensor.matmul(ps_o[:],
                                     v[:, 2 * g + jc, m * P:(m + 1) * P],
                                     attnT[:, jc, :],
                                     start=(jc == 0), stop=(jc == 1))
                nc.vector.tensor_copy(out=o[:, m, g * GT:(g + 1) * GT],
                                      in_=ps_o[:])

        # ---- out-proj + residual
        for m in range(DC):
            w = load_wcol(wout_d, li, m, DC)
            ps = ps_pool.tile([P, TLOC], FP32, tag="mm", bufs=4)
            for ec in range(DC):
                nc.tensor.matmul(ps[:], w[:, ec, :], o[:, ec, :],
                                 start=(ec == 0), stop=(ec == DC - 1))
            nc.vector.scalar_tensor_tensor(
                out=h[:, m, :], in0=ps[:], scalar=woutb[:, li, m:m + 1],
                in1=h[:, m, :], op0=ALU.add, op1=ALU.add)

        # ---- LN2 -> y2
        y2 = acts.tile([P, DC, TLOC], BF16, tag="y")
        layernorm(li, ln2g, ln2b, y2)

        # ---- ff1 + LeakyReLU -> z
        z = acts.tile([P, FC, TLOC], BF16, tag="z")
        for m in range(FC):
            w = load_wcol(ff1_d, li, m, DC)
            ps = ps_pool.tile([P, TLOC], FP32, tag="mm", bufs=4)
            for kc in range(DC):
                nc.tensor.matmul(ps[:], w[:, kc, :], y2[:, kc, :],
                                 start=(kc == 0), stop=(kc == DC - 1))
            t_ff = lnp.tile([P, TLOC], FP32, tag="t_ff", bufs=2)
            nc.scalar.activation(out=t_ff[:], in_=ps[:], func=AF.Identity,
                                 bias=ff1b[:, li, m:m + 1])
            nc.vector.scalar_tensor_tensor(
                out=z[:, m, :], in0=t_ff[:], scalar=NEG_SLOPE, in1=t_ff[:],
                op0=ALU.mult, op1=ALU.max)

        # ---- ff2 + residual
        for m in range(DC):
            w = load_wcol(ff2_d, li, m, FC)
            ps = ps_pool.tile([P, TLOC], FP32, tag="mm", bufs=4)
            for fc in range(FC):
                nc.tensor.matmul(ps[:], w[:, fc, :], z[:, fc, :],
                                 start=(fc == 0), stop=(fc == FC - 1))
            nc.vector.scalar_tensor_tensor(
                out=h[:, m, :], in0=ps[:], scalar=ff2b[:, li, m:m + 1],
                in1=h[:, m, :], op0=ALU.add, op1=ALU.add)

    # close layer-phase pools (LIFO)
    lnp.release()
    ps_pool.release()
    wst.release()
    acts.release()

    # ---------------- logits: out = h @ logit_w + logit_b ------------------
    out_d = nc._dram["out"]
    wl_d = nc._dram["wl"]
    lb_d = nc._dram["lb"]
    with tc.tile_pool(name="lg", bufs=1) as lg, \
         tc.tile_pool(name="lgps", bufs=1, space="PSUM") as lgps:
        hl = lg.tile([P, DC, TLOC], BF16, name="hl")
        for dc in range(DC):
            nc.scalar.copy(out=hl[:, dc, :], in_=h[:, dc, :])
        for vs in range(NVS):
            wls = lg.tile([P, DC, VS], BF16, tag="wls", bufs=2)
            # per-kc DMA so the first matmuls of the super start after 1/8
            # of the weight tile has landed
            for dc in range(DC):
                nc.scalar.dma_start(
                    wls[:, dc, :],
                    wl_d.ap()[dc * P:(dc + 1) * P, vs * VS:(vs + 1) * VS])
            bias_bc = lg.tile([P, VS], FP32, tag="bias_bc", bufs=2)
            nc.scalar.dma_start(bias_bc[:], bass.AP(
                tensor=lb_d, offset=vs * VS, ap=[[0, P], [1, VS]]))
            for tt in range(TT):
                pss = [lgps.tile([P, 512], FP32, tag="lgmm", bufs=8,
                                 name=f"lgmm{vs}_{tt}_{nb}")
                       for nb in range(4)]
                for kc in range(DC):
                    for nb in range(4):
                        nc.tensor.matmul(
                            pss[nb][:, :500], hl[:, kc, tt * P:(tt + 1) * P],
                            wls[:, kc, nb * 500:(nb + 1) * 500],
                            start=(kc == 0), stop=(kc == DC - 1))
                lsb = lg.tile([P, VS], FP32, tag="lsb", bufs=4)
                for nb in range(4):
                    nc.vector.tensor_add(
                        out=lsb[:, nb * 500:(nb + 1) * 500],
                        in0=pss[nb][:, :500],
                        in1=bias_bc[:, nb * 500:(nb + 1) * 500])
                nc.scalar.dma_start(
                    out_d.ap()[tt * P:(tt + 1) * P, vs * VS:(vs + 1) * VS],
                    lsb[:])

    resid.release()
    singles.release()


def build_kernel():
    nc = bacc.Bacc(num_devices=NC)
    nc._dram = {}

    def din(name, shape, dt):
        nc._dram[name] = nc.dram_tensor(name, shape, dt, kind="ExternalInput")

    din("idx", [P, TLOC // 16], I16)
    din("emb", [V, D], FP32)
    din("wqkv", [L, 3 * DC, P, DC, P], BF16)
    din("wout", [L, DC, P, DC, P], BF16)
    din("ff1w", [L, FC, P, DC, P], BF16)
    din("ff2w", [L, DC, P, FC, P], BF16)
    din("wl", [D, V], BF16)
    din("lb", [V], FP32)
    for nm, dim in (("ln1_g", D), ("ln1_b", D), ("ln2_g", D), ("ln2_b", D),
                    ("wout_b", D), ("ff2_b", D)):
        din(nm, [L, dim], FP32)
    din("ff1_b", [L, 4 * D], FP32)
    nc._dram["out"] = nc.dram_tensor("out", [TLOC, V], FP32,
                                     kind="ExternalOutput")

    with tile.TileContext(nc) as tc:
        _emit(nc, tc)
    nc.finalize()
    return nc


# ---------------------------------------------------------------- host side
def _to_bf16(a):
    return np.asarray(a, np.float32).astype(ml_dtypes.bfloat16)


def prep_inputs(inputs):
    """Full inputs -> (shared per-core dict, list of per-core idx arrays)."""
    x = np.asarray(inputs["x"])
    assert x.shape == (B, T)
    def _swz(w, kc, ncol):
        # [L, K, Dout] -> [L, col, p, c, m]: per-(col, p) contiguous kc*128
        return np.ascontiguousarray(
            _to_bf16(w).reshape(L, kc, P, ncol, P).transpose(0, 3, 2, 1, 4))

    wqkv = _swz(inputs["wqkv"], DC, 3 * DC)
    wout = _swz(inputs["wout_w"], DC, DC)
    ff1 = _swz(inputs["ff1_w"], DC, FC)
    ff2 = _swz(inputs["ff2_w"], FC, DC)
    shared = {
        "emb": np.asarray(inputs["token_emb"], np.float32),
        "wqkv": wqkv,
        "wout": wout,
        "ff1w": ff1,
        "ff2w": ff2,
        "wl": _to_bf16(inputs["logit_w"]),
        "lb": np.asarray(inputs["logit_b"], np.float32),
        "ln1_g": np.asarray(inputs["ln1_g"], np.float32),
        "ln1_b": np.asarray(inputs["ln1_b"], np.float32),
        "ln2_g": np.asarray(inputs["ln2_g"], np.float32),
        "ln2_b": np.asarray(inputs["ln2_b"], np.float32),
        "wout_b": np.asarray(inputs["wout_b"], np.float32),
        "ff2_b": np.asarray(inputs["ff2_b"], np.float32),
        "ff1_b": np.asarray(inputs["ff1_b"], np.float32),
    }
    idxs = []
    for c in range(NC):
        ids = x[:, c::H].reshape(-1).astype(np.int16)  # [512], b-major
        wrapped = ids.reshape(TLOC // 16, 16).T        # [16, 32]
        idxs.append(np.ascontiguousarray(np.tile(wrapped, (8, 1))))  # [128, 32]
    return shared, idxs


def assemble_output(per_core):
    """8 x [512, V] (token order: b, i) -> [B, T, V] with t = i*H + c."""
    arr = np.stack(per_core)                   # [8, 2, 256, V] after reshape
    arr = arr.reshape(NC, B, GT, V)
    return np.ascontiguousarray(arr.transpose(1, 2, 0, 3).reshape(B, T, V))


def kernel(**inputs):
    nc = _CACHE.get("nc")
    if nc is None:
        nc = _CACHE["nc"] = build_kernel()
    shared, idxs = prep_inputs(inputs)
    in_maps = [dict(shared, idx=idxs[c]) for c in range(NC)]
    res = run_bass_kernel_spmd(nc, in_maps, core_ids=list(range(NC)))
    _CACHE["last_result"] = res
    return assemble_output([res.results[c]["out"] for c in range(NC)])



# revision 3
# speedup vs baseline: 102.8074x; 102.8074x over previous
"""Trainium2 Bass kernel for a 4-layer dense transformer (CompressiveTransformer).

Model: V=32000, D=1024, L=4, H=8, B=2, T=2048.

Sharding: the reference attention reshapes (B, T//H, H, D) -> heads are the
strided token residue classes t mod 8.  With 8 cores, core c owns tokens
{t : t % 8 == c} of both batch rows (512 tokens).  Attention, layernorm, FFN
and logits are then all fully core-local: no collectives.

On-chip layout: the residual stream h is kept feature-major
[128 part, 8 dchunk, 512 tok] fp32 in SBUF for the whole kernel.  Matmuls run
in bf16 (fp32 PSUM accumulation); weights are cast to bf16 on the host and
uploaded in column-blocked layout so weight-tile DMAs are contiguous.
"""

import numpy as np
import ml_dtypes

import concourse.bass as bass
import concourse.mybir as mybir
import concourse.tile as tile
from concourse import bacc
from concourse.bass_utils import run_bass_kernel_spmd
from concourse.masks import make_identity

# ---------------------------------------------------------------- constants
V = 32000
D = 1024
L = 4
H = 8
B = 2
T = 2048
NC = 8                     # cores
TLOC = B * T // NC         # 512 tokens per core
P = 128
DC = D // P                # 8 feature chunks
FC = 4 * D // P            # 32 ff chunks
TT = TLOC // P             # 4 token tiles per core
GRP = B                    # attention groups per core (one per batch row)
GT = T // H                # 256 tokens per attention group
VS = 2000                  # vocab super-chunk
NVS = V // VS              # 16
SCALE = float((D // H) ** -0.5)
EPS = 1e-5
NEG_SLOPE = 0.01

FP32 = mybir.dt.float32
BF16 = mybir.dt.bfloat16
I16 = mybir.dt.int16
AF = mybir.ActivationFunctionType
ALU = mybir.AluOpType
AX = mybir.AxisListType

_CACHE = {}


# ---------------------------------------------------------------- emission
def _emit(nc, tc, rep=""):
    singles = tc.alloc_tile_pool(name=f"singles{rep}", bufs=1)
    ident_f32 = singles.tile([P, P], FP32, name="ident_f32")
    make_identity(nc, ident_f32)
    ident_bf = singles.tile([P, P], BF16, name="ident_bf")
    make_identity(nc, ident_bf)
    ones_col = singles.tile([P, 1], BF16, name="ones_col")  # lhsT for colsums
    nc.vector.memset(ones_col, 1.0)
    eps_sb = singles.tile([1, 1], FP32, name="eps_sb")
    nc.vector.memset(eps_sb, EPS)

    # per-feature params, fm layout [128, L, DC]
    def param_sb(name, dram, nchunk):
        t = singles.tile([P, L, nchunk], FP32, name=name)
        nc.sync.dma_start(t[:], dram.ap().rearrange("l (c p) -> p l c", p=P))
        return t

    ln1g = param_sb("ln1g", nc._dram["ln1_g"], DC)
    ln1b = param_sb("ln1b", nc._dram["ln1_b"], DC)
    ln2g = param_sb("ln2g", nc._dram["ln2_g"], DC)
    ln2b = param_sb("ln2b", nc._dram["ln2_b"], DC)
    woutb = param_sb("woutb", nc._dram["wout_b"], DC)
    ff2b = param_sb("ff2b", nc._dram["ff2_b"], DC)
    ff1b = param_sb("ff1b", nc._dram["ff1_b"], FC)

    idx_sb = singles.tile([P, TLOC // 16], I16, name="idx_sb")
    nc.sync.dma_start(idx_sb[:], nc._dram["idx"].ap())

    resid = tc.alloc_tile_pool(name=f"resid{rep}", bufs=1)
    h = resid.tile([P, DC, TLOC], FP32, name="h")

    # ---------------- phase 0: embedding gather + transpose to fm ----------
    with tc.tile_pool(name=f"embp{rep}", bufs=1) as embp, \
         tc.tile_pool(name=f"embps{rep}", bufs=4, space="PSUM") as embps:
        emb_tm = embp.tile([P, TT, D], FP32, name="emb_tm")
        nc.gpsimd.dma_gather(
            out_ap=emb_tm[:],
            in_ap=nc._dram["emb"].ap(),
            idxs_ap=idx_sb[:],
            num_idxs=TLOC,
            num_idxs_reg=TLOC,
            elem_size=D,
        )
        for c in range(TT):
            for dc in range(DC):
                ps = embps.tile([P, P], FP32, tag="tp")
                nc.tensor.transpose(ps[:], emb_tm[:, c, dc * P:(dc + 1) * P],
                                    ident_f32[:])
                nc.vector.tensor_copy(out=h[:, dc, c * P:(c + 1) * P], in_=ps[:])

    # ---------------- transformer layers ----------------------------------
    acts = tc.alloc_tile_pool(name=f"acts{rep}", bufs=1)
    wst = tc.alloc_tile_pool(name=f"wst{rep}", bufs=1)
    ps_pool = tc.alloc_tile_pool(name=f"ps{rep}", bufs=1, space="PSUM")
    lnp = tc.alloc_tile_pool(name=f"lnp{rep}", bufs=1)

    def layernorm(li, gain, bias_, y):
        """y = LN(h) * gain + bias  (bf16, fm layout), stats over features."""
        hb = lnp.tile([P, DC, TLOC], BF16, tag="hb")
        h2 = lnp.tile([P, DC, TLOC], BF16, tag="h2")
        # per-chunk casts so colsums can start while the previous matmul
        # phase is still finalizing other h chunks
        for dc in range(DC):
            nc.scalar.copy(out=hb[:, dc, :], in_=h[:, dc, :])
            nc.vector.tensor_mul(out=h2[:, dc, :], in0=h[:, dc, :],
                                 in1=h[:, dc, :])
        ps_s = ps_pool.tile([1, TLOC], FP32, tag="stat", bufs=2)
        ps_s2 = ps_pool.tile([1, TLOC], FP32, tag="stat", bufs=2)
        for dc in range(DC):
            nc.tensor.matmul(ps_s[:], ones_col[:], hb[:, dc, :],
                             start=(dc == 0), stop=(dc == DC - 1))
        for dc in range(DC):
            nc.tensor.matmul(ps_s2[:], ones_col[:], h2[:, dc, :],
                             start=(dc == 0), stop=(dc == DC - 1))
        mr = lnp.tile([1, 2, TLOC], FP32, tag="mr", bufs=1)  # mu | rs packed
        nc.vector.tensor_scalar_mul(mr[:, 0, :], ps_s[:], 1.0 / D)
        mumu = lnp.tile([1, TLOC], FP32, tag="mumu", bufs=1)
        nc.vector.tensor_mul(out=mumu[:], in0=mr[:, 0, :], in1=mr[:, 0, :])
        var = lnp.tile([1, TLOC], FP32, tag="var", bufs=1)
        nc.vector.scalar_tensor_tensor(
            out=var[:], in0=ps_s2[:], scalar=1.0 / D, in1=mumu[:],
            op0=ALU.mult, op1=ALU.subtract)
        lnv = lnp.tile([1, TLOC], FP32, tag="lnv", bufs=1)
        nc.scalar.activation(out=lnv[:], in_=var[:], func=AF.Ln, bias=eps_sb[:])
        nc.scalar.activation(out=mr[:, 1, :], in_=lnv[:], func=AF.Exp,
                             scale=-0.5)
        mr_b = lnp.tile([P, 2, TLOC], FP32, tag="mr_b", bufs=1)
        nc.gpsimd.partition_broadcast(mr_b[:], mr[:])
        for dc in range(DC):
            t1 = lnp.tile([P, TLOC], FP32, tag="t1", bufs=2)
            nc.vector.tensor_sub(out=t1[:], in0=h[:, dc, :], in1=mr_b[:, 0, :])
            t2 = lnp.tile([P, TLOC], FP32, tag="t2", bufs=2)
            nc.vector.tensor_mul(out=t2[:], in0=t1[:], in1=mr_b[:, 1, :])
            nc.scalar.activation(
                out=y[:, dc, :], in_=t2[:], func=AF.Identity,
                scale=gain[:, li, dc:dc + 1], bias=bias_[:, li, dc:dc + 1])

    wqkv_d = nc._dram["wqkv"]   # [L, 24, D, 128] blocked bf16
    wout_d = nc._dram["wout"]   # [L, 8, D, 128]
    ff1_d = nc._dram["ff1w"]    # [L, 32, D, 128]
    ff2_d = nc._dram["ff2w"]    # [L, 8, 4D, 128]

    def load_wcol(dram, li, col, kc):
        """Host-swizzled weight column [P, kc, 128] -> SBUF, contiguous."""
        t = wst.tile([P, kc, P], BF16, tag=f"wcol{kc}", bufs=4 if kc == DC else 3)
        nc.sync.dma_start(t[:], dram.ap()[li, col])
        return t

    for li in range(L):
        # ---- LN1 -> y
        y = acts.tile([P, DC, TLOC], BF16, tag="y")
        layernorm(li, ln1g, ln1b, y)

        # ---- q, k fm via weight-stationary matmuls
        q = acts.tile([P, DC, TLOC], BF16, tag="q")
        k = acts.tile([P, DC, TLOC], BF16, tag="k")
        for dst, base in ((q, 0), (k, DC)):
            for m in range(DC):
                w = load_wcol(wqkv_d, li, base + m, DC)
                ps = ps_pool.tile([P, TLOC], FP32, tag="mm", bufs=4)
                for kc in range(DC):
                    nc.tensor.matmul(ps[:], w[:, kc, :], y[:, kc, :],
                                     start=(kc == 0), stop=(kc == DC - 1))
                nc.vector.tensor_copy(out=dst[:, m, :], in_=ps[:])

        # ---- v token-major via activation-stationary matmuls
        v = acts.tile([P, TT, D], BF16, tag="v")
        for half in range(2):
            wv = wst.tile([P, DC, 512], BF16, tag="wv", bufs=2)
            for cb in range(4):
                nc.sync.dma_start(
                    wv[:, :, cb * P:(cb + 1) * P],
                    wqkv_d.ap()[li, 2 * DC + 4 * half + cb])
            for tt in range(TT):
                ps = ps_pool.tile([P, 512], FP32, tag="mm", bufs=4)
                for kc in range(DC):
                    nc.tensor.matmul(ps[:], y[:, kc, tt * P:(tt + 1) * P],
                                     wv[:, kc, :],
                                     start=(kc == 0), stop=(kc == DC - 1))
                nc.vector.tensor_copy(
                    out=v[:, tt, half * 512:(half + 1) * 512], in_=ps[:])

        # ---- attention per group (fully local, 256 tokens, head dim D)
        o = acts.tile([P, DC, TLOC], BF16, tag="o")
        for g in range(GRP):
            attnT = acts.tile([P, 2, GT], BF16, tag="attnT")
            for it in range(2):
                ps_d = ps_pool.tile([P, GT], FP32, tag="mm", bufs=4)
                for kc in range(DC):
                    nc.tensor.matmul(
                        ps_d[:], q[:, kc, (2 * g + it) * P:(2 * g + it + 1) * P],
                        k[:, kc, g * GT:(g + 1) * GT],
                        start=(kc == 0), stop=(kc == DC - 1))
                mx = lnp.tile([P, 1], FP32, tag="mx", bufs=2)
                nc.vector.reduce_max(mx[:], ps_d[:], axis=AX.X)
                nmx = lnp.tile([P, 1], FP32, tag="nmx", bufs=2)
                nc.vector.tensor_scalar_mul(nmx[:], mx[:], -SCALE)
                ae = lnp.tile([P, GT], BF16, tag="ae", bufs=2)
                se = lnp.tile([P, 1], FP32, tag="se", bufs=2)
                nc.scalar.activation(out=ae[:], in_=ps_d[:], func=AF.Exp,
                                     bias=nmx[:], scale=SCALE, accum_out=se[:])
                rse = lnp.tile([P, 1], FP32, tag="rse", bufs=2)
                nc.vector.reciprocal(out=rse[:], in_=se[:])
                an = lnp.tile([P, GT], BF16, tag="an", bufs=2)
                nc.vector.tensor_scalar_mul(an[:], ae[:], rse[:])
                for jc in range(2):
                    ps_t = ps_pool.tile([P, P], BF16, tag="tp", bufs=2)
                    nc.tensor.transpose(ps_t[:], an[:, jc * P:(jc + 1) * P],
                                        ident_bf[:])
                    nc.vector.tensor_copy(out=attnT[:, jc, it * P:(it + 1) * P],
                                          in_=ps_t[:])
            for m in range(DC):
                ps_o = ps_pool.tile([P, GT], FP32, tag="mm", bufs=4)
                for jc in range(2):
                    nc.tensor.matmul(ps_o[:],
                                     v[:, 2 * g + jc, m * P:(m + 1) * P],
                                     attnT[:, jc, :],
                                     start=(jc == 0), stop=(jc == 1))
                nc.vector.tensor_copy(out=o[:, m, g * GT:(g + 1) * GT],
                                      in_=ps_o[:])

        # ---- out-proj + residual
        for m in range(DC):
            w = load_wcol(wout_d, li, m, DC)
            ps = ps_pool.tile([P, TLOC], FP32, tag="mm", bufs=4)
            for ec in range(DC):
                nc.tensor.matmul(ps[:], w[:, ec, :], o[:, ec, :],
                                 start=(ec == 0), stop=(ec == DC - 1))
            nc.vector.scalar_tensor_tensor(
                out=h[:, m, :], in0=ps[:], scalar=woutb[:, li, m:m + 1],
                in1=h[:, m, :], op0=ALU.add, op1=ALU.add)

        # ---- LN2 -> y2
        y2 = acts.tile([P, DC, TLOC], BF16, tag="y")
        layernorm(li, ln2g, ln2b, y2)

        # ---- ff1 + LeakyReLU -> z
        z = acts.tile([P, FC, TLOC], BF16, tag="z")
        for m in range(FC):
            w = load_wcol(ff1_d, li, m, DC)
            ps = ps_pool.tile([P, TLOC], FP32, tag="mm", bufs=4)
            for kc in range(DC):
                nc.tensor.matmul(ps[:], w[:, kc, :], y2[:, kc, :],
                                 start=(kc == 0), stop=(kc == DC - 1))
            t_ff = lnp.tile([P, TLOC], FP32, tag="t_ff", bufs=2)
            nc.scalar.activation(out=t_ff[:], in_=ps[:], func=AF.Identity,
                                 bias=ff1b[:, li, m:m + 1])
            nc.vector.scalar_tensor_tensor(
                out=z[:, m, :], in0=t_ff[:], scalar=NEG_SLOPE, in1=t_ff[:],
                op0=ALU.mult, op1=ALU.max)

        # ---- ff2 + residual
        for m in range(DC):
            w = load_wcol(ff2_d, li, m, FC)
            ps = ps_pool.tile([P, TLOC], FP32, tag="mm", bufs=4)
            for fc in range(FC):
                nc.tensor.matmul(ps[:], w[:, fc, :], z[:, fc, :],
                                 start=(fc == 0), stop=(fc == FC - 1))
            nc.vector.scalar_tensor_tensor(
                out=h[:, m, :], in0=ps[:], scalar=ff2b[:, li, m:m + 1],
                in1=h[:, m, :], op0=ALU.add, op1=ALU.add)

    # close layer-phase pools (LIFO)
    lnp.release()
    ps_pool.release()
    wst.release()
    acts.release()

    # ---------------- logits: out = h @ logit_w + logit_b ------------------
    out_d = nc._dram["out"]
    wl_d = nc._dram["wl"]
    lb_d = nc._dram["lb"]
    with tc.tile_pool(name=f"lg{rep}", bufs=1) as lg, \
         tc.tile_pool(name=f"lgps{rep}", bufs=1, space="PSUM") as lgps:
        hl = lg.tile([P, DC, TLOC], BF16, name="hl")
        for dc in range(DC):
            nc.scalar.copy(out=hl[:, dc, :], in_=h[:, dc, :])
        for vs in range(NVS):
            wls = lg.tile([P, DC, VS], BF16, tag="wls", bufs=2)
            # per-kc DMA so the first matmuls of the super start after 1/8
            # of the weight tile has landed
            for dc in range(DC):
                nc.scalar.dma_start(
                    wls[:, dc, :],
                    wl_d.ap()[dc * P:(dc + 1) * P, vs * VS:(vs + 1) * VS])
            bias_bc = lg.tile([P, VS], FP32, tag="bias_bc", bufs=2)
            nc.scalar.dma_start(bias_bc[:], bass.AP(
                tensor=lb_d, offset=vs * VS, ap=[[0, P], [1, VS]]))
            for tt in range(TT):
                pss = [lgps.tile([P, 512], FP32, tag="lgmm", bufs=8,
                                 name=f"lgmm{vs}_{tt}_{nb}")
                       for nb in range(4)]
                for kc in range(DC):
                    for nb in range(4):
                        nc.tensor.matmul(
                            pss[nb][:, :500], hl[:, kc, tt * P:(tt + 1) * P],
                            wls[:, kc, nb * 500:(nb + 1) * 500],
                            start=(kc == 0), stop=(kc == DC - 1))
                lsb = lg.tile([P, VS], FP32, tag="lsb", bufs=4)
                for nb in range(4):
                    nc.vector.tensor_add(
                        out=lsb[:, nb * 500:(nb + 1) * 500],
                        in0=pss[nb][:, :500],
                        in1=bias_bc[:, nb * 500:(nb + 1) * 500])
                nc.scalar.dma_start(
                    out_d.ap()[tt * P:(tt + 1) * P, vs * VS:(vs + 1) * VS],
                    lsb[:])

    resid.release()
    singles.release()


def build_kernel(reps=1):
    nc = bacc.Bacc(num_devices=NC)
    nc._dram = {}

    def din(name, shape, dt):
        nc._dram[name] = nc.dram_tensor(name, shape, dt, kind="ExternalInput")

    din("idx", [P, TLOC // 16], I16)
    din("emb", [V, D], FP32)
    din("wqkv", [L, 3 * DC, P, DC, P], BF16)
    din("wout", [L, DC, P, DC, P], BF16)
    din("ff1w", [L, FC, P, DC, P], BF16)
    din("ff2w", [L, DC, P, FC, P], BF16)
    din("wl", [D, V], BF16)
    din("lb", [V], FP32)
    for nm, dim in (("ln1_g", D), ("ln1_b", D), ("ln2_g", D), ("ln2_b", D),
                    ("wout_b", D), ("ff2_b", D)):
        din(nm, [L, dim], FP32)
    din("ff1_b", [L, 4 * D], FP32)
    nc._dram["out"] = nc.dram_tensor("out", [TLOC, V], FP32,
                                     kind="ExternalOutput")

    with tile.TileContext(nc) as tc:
        for r in range(reps):
            _emit(nc, tc, rep="" if r == 0 else f"_r{r}")
    nc.finalize()
    return nc


# ---------------------------------------------------------------- host side
def _to_bf16(a):
    return np.asarray(a, np.float32).astype(ml_dtypes.bfloat16)


def prep_inputs(inputs):
    """Full inputs -> (shared per-core dict, list of per-core idx arrays)."""
    x = np.asarray(inputs["x"])
    assert x.shape == (B, T)
    def _swz(w, kc, ncol):
        # [L, K, Dout] -> [L, col, p, c, m]: per-(col, p) contiguous kc*128
        return np.ascontiguousarray(
            _to_bf16(w).reshape(L, kc, P, ncol, P).transpose(0, 3, 2, 1, 4))

    wqkv = _swz(inputs["wqkv"], DC, 3 * DC)
    wout = _swz(inputs["wout_w"], DC, DC)
    ff1 = _swz(inputs["ff1_w"], DC, FC)
    ff2 = _swz(inputs["ff2_w"], FC, DC)
    shared = {
        "emb": np.asarray(inputs["token_emb"], np.float32),
        "wqkv": wqkv,
        "wout": wout,
        "ff1w": ff1,
        "ff2w": ff2,
        "wl": _to_bf16(inputs["logit_w"]),
        "lb": np.asarray(inputs["logit_b"], np.float32),
        "ln1_g": np.asarray(inputs["ln1_g"], np.float32),
        "ln1_b": np.asarray(inputs["ln1_b"], np.float32),
        "ln2_g": np.asarray(inputs["ln2_g"], np.float32),
        "ln2_b": np.asarray(inputs["ln2_b"], np.float32),
        "wout_b": np.asarray(inputs["wout_b"], np.float32),
        "ff2_b": np.asarray(inputs["ff2_b"], np.float32),
        "ff1_b": np.asarray(inputs["ff1_b"], np.float32),
    }
    idxs = []
    for c in range(NC):
        ids = x[:, c::H].reshape(-1).astype(np.int16)  # [512], b-major
        wrapped = ids.reshape(TLOC // 16, 16).T        # [16, 32]
        idxs.append(np.ascontiguousarray(np.tile(wrapped, (8, 1))))  # [128, 32]
    return shared, idxs


def assemble_output(per_core):
    """8 x [512, V] (token order: b, i) -> [B, T, V] with t = i*H + c."""
    arr = np.stack(per_core)                   # [8, 2, 256, V] after reshape
    arr = arr.reshape(NC, B, GT, V)
    return np.ascontiguousarray(arr.transpose(1, 2, 0, 3).reshape(B, T, V))


def kernel(**inputs):
    nc = _CACHE.get("nc")
    if nc is None:
        nc = _CACHE["nc"] = build_kernel()
    shared, idxs = prep_inputs(inputs)
    in_maps = [dict(shared, idx=idxs[c]) for c in range(NC)]
    res = run_bass_kernel_spmd(nc, in_maps, core_ids=list(range(NC)))
    _CACHE["last_result"] = res
    return assemble_output([res.results[c]["out"] for c in range(NC)])

